# revision 75
# baseline (speedup 1.0000x reference)
"""Trainium2 Bass kernel for causal self-attention with RoPE (fp8 attention).

Model: x[4,2048,1024] -> qkv = x@Wqkv -> RoPE(q,k) -> causal SDPA -> out@Wout.

Sharding (8 cores): core c handles batch b=c//2, head-group g=c%2 (8 of 16
heads). Each core computes a partial output over its 512 features of Wout
rows; the host sums the two partials per batch.

Design (cost model: matmul cost = moving-dim size x cycles/row; fp8
DoubleRow = 0.5 c/r; bf16 = 1 c/r; contraction width and stationary loads
are free):
  - qkv projection in bf16; psum evicted to bf16 RoPE tiles on DVE.
  - RoPE: rotate-half via 4 partition-shifted SBUF DMAs (q+k fused in one
    tile), bf16 muls + adds on DVE (2x mode) writing qT/kTb directly.
  - scores: plain bf16 matmuls (64-contraction, [k,q]^T tiles, causal
    lower bounds per k-tile).
  - causal masking on the PE: fp8 DoubleRow [I|0] @ (-240 triangle)
    accumulated into the scores psum; exp(s/8 - 2) into fp8 underflows
    masked entries to exact zero (psum start=True arms a bank-wide
    pending-zero, so the gap-mask lands on zeros; -2 bias keeps the max
    softmax weight under fp8e4's 240 max).
  - exp on ACT writes the fp8 `at` pair tile [128, 2, 512] directly; the
    ones column of V_aug yields softmax denominators in psum col 64.
  - attn@V via fp8 DoubleRow over k-tile pairs: lhsT = at[:, :, qc-block],
    rhs = v_sb[:, j:j+2, h, :]; 32.5 cycles per pair, with a second
    accumulating pass over v_lo (fp8 residual of V) for accuracy.
  - AV psum [128, 4qc, 65] shares one bank; only the first matmul uses
    start=True (the armed pending-zero zeroes the other qc slots).
  - normalization during AV-psum eviction: strided reciprocal of psum col
    64 + per-qc tensor_scalar_mul into bf16 avn tiles.
  - attn^T built by PE pair-transposes (2 heads per [128,128] tile, bf16
    identity) and kept SBUF-resident; no DRAM bounce.
  - output projection in bf16 from the resident attn^T.
  - schedule: phase-1 spans, attention pairs (h-major, 2-pair produce
    lookahead), previous-span transposes, and the output projection are
    woven at fine grain so PE/ACT/DVE overlap.
"""

import os
import sys

import numpy as np


def _import_concourse():
    try:
        import concourse  # noqa: F401
    except ImportError:
        for p in ("/opt/trn_rl_repo", "/root/.axon_site/_ro/trn_rl_repo"):
            if os.path.isdir(p) and p not in sys.path:
                sys.path.insert(0, p)
        import concourse  # noqa: F401


_import_concourse()

import concourse.bacc as bacc
import concourse.bass as bass
import concourse.mybir as mybir
import concourse.tile as tile
from concourse.bass_utils import run_bass_kernel_spmd

# ---------------------------------------------------------------------------
D_MODEL = 1024
N_HEADS = 16
HEAD_DIM = 64
ROPE_BASE = 10000.0
BATCH = 4
T_FULL = 2048
N_CORES = 8

HPC = 8                 # heads per core
FEAT = HPC * HEAD_DIM   # 512
DCH = D_MODEL // 128    # 8 contraction chunks
NFB = FEAT // 128       # 4 feature blocks

F32 = mybir.dt.float32
BF16 = mybir.dt.bfloat16
FP8 = mybir.dt.float8e4
DR = mybir.MatmulPerfMode.DoubleRow

MASKVAL = -240.0        # fp8-exact; exp((s-240)/8-2) underflows to exact 0
EXP_BIAS = -2.0         # keeps max softmax weight < fp8 max (240)


def build_nc(T=T_FULL, debug=False):
    SPAN1 = 256
    NSPAN1 = T // SPAN1
    SPAN2 = 512
    NSPAN2 = T // SPAN2
    NTOK = T // 128

    nc = bacc.Bacc(None, target_bir_lowering=False)

    dbg = {}
    if debug:
        dbg["qT"] = nc.dram_tensor("d_qT", [128, NFB, T], BF16, kind="ExternalOutput")
        dbg["kT"] = nc.dram_tensor("d_kT", [128, NFB, T], BF16, kind="ExternalOutput")
        dbg["v"] = nc.dram_tensor("d_v", [128, NTOK, HPC, HEAD_DIM + 1], FP8,
                                  kind="ExternalOutput")
        dbg["at"] = nc.dram_tensor("d_at", [128, 2, 512], FP8, kind="ExternalOutput")
        dbg["at2"] = nc.dram_tensor("d_at2", [128, 2, 512], FP8, kind="ExternalOutput")
        dbg["ps"] = nc.dram_tensor("d_ps", [128, 2, 512], F32, kind="ExternalOutput")
        dbg["avn"] = nc.dram_tensor("d_avn", [128, 4, HPC, HEAD_DIM], BF16,
                                    kind="ExternalOutput")
        dbg["attnT"] = nc.dram_tensor("d_attnT", [128, NFB, T], BF16,
                                      kind="ExternalOutput")

    xt_d = nc.dram_tensor("xt", [D_MODEL, T], BF16, kind="ExternalInput")
    wq_d = nc.dram_tensor("wq", [D_MODEL, FEAT], BF16, kind="ExternalInput")
    wk_d = nc.dram_tensor("wk", [D_MODEL, FEAT], BF16, kind="ExternalInput")
    wv_d = nc.dram_tensor("wv", [D_MODEL, FEAT], BF16, kind="ExternalInput")
    wo_d = nc.dram_tensor("wo", [FEAT, D_MODEL], BF16, kind="ExternalInput")
    cs_d = nc.dram_tensor("cs", [128, T], BF16, kind="ExternalInput")
    sn_d = nc.dram_tensor("sn", [128, T], BF16, kind="ExternalInput")
    tri_d = nc.dram_tensor("tri", [128, 128], FP8, kind="ExternalInput")
    trg_d = nc.dram_tensor("trg", [128, 128], FP8, kind="ExternalInput")
    id8_d = nc.dram_tensor("id8", [128, 2, 128], FP8, kind="ExternalInput")
    idb_d = nc.dram_tensor("idb", [128, 128], BF16, kind="ExternalInput")
    out_d = nc.dram_tensor("out", [T, D_MODEL], F32, kind="ExternalOutput")

    with tile.TileContext(nc) as tc:
        pools = []

        def pool(name, bufs, space="SBUF"):
            p = tc.alloc_tile_pool(name=name, bufs=bufs, space=space)
            pools.append(p)
            return p

        # ---- persistent tensors --------------------------------------
        pbig = pool("big", 1)
        qT = pbig.tile([128, NFB, T], BF16, name="qT")
        kTb = pbig.tile([128, NFB, T], BF16, name="kTb")
        v_sb = pbig.tile([128, NTOK, HPC, HEAD_DIM + 1], FP8, name="v_sb")
        v_lo = pbig.tile([128, NTOK, HPC, HEAD_DIM + 1], FP8, name="v_lo")
        attnT = pbig.tile([128, NFB, T], BF16, name="attnT")
        wq_sb = pbig.tile([128, DCH, FEAT], BF16, name="wq_sb")
        wk_sb = pbig.tile([128, DCH, FEAT], BF16, name="wk_sb")
        wv_sb = pbig.tile([128, DCH, FEAT], BF16, name="wv_sb")
        wo_sb = pbig.tile([128, NFB, D_MODEL], BF16, name="wo_sb")
        cs_sb = pbig.tile([128, T], BF16, name="cs_sb")
        sn_sb = pbig.tile([128, T], BF16, name="sn_sb")
        tri_sb = pbig.tile([128, 128], FP8, name="tri_sb")
        trg_sb = pbig.tile([128, 128], FP8, name="trg_sb")
        id8_sb = pbig.tile([128, 2, 128], FP8, name="id8_sb")
        idb_sb = pbig.tile([128, 128], BF16, name="idb_sb")
        bias_sb = pbig.tile([128, 1], F32, name="bias_sb")

        # ones column of V_aug (zeros in v_lo's), exp bias
        nc.gpsimd.memset(v_sb[:, :, :, HEAD_DIM], 1.0)
        nc.gpsimd.memset(v_lo[:, :, :, HEAD_DIM], 0.0)
        nc.vector.memset(bias_sb[:], EXP_BIAS)

        # ---- pools ------------------------------------------------------
        pscore = pool("pscore", 2, space="PSUM")   # [128,2,512] f32: 2 banks
        pav = pool("pav", 2, space="PSUM")         # [128,4,65] f32: 1 bank
        pwork = pool("pwork", 2, space="PSUM")     # 1-bank work tiles

        p1x = pool("p1x", 2)
        prope = pool("prope", 4)
        pat = pool("pat", 3)
        pavn = pool("pavn", 2)
        psc = pool("psc", 4)
        pot = pool("pot", 6)

        xt_view = xt_d[:].rearrange("(c p) t -> p c t", p=128)

        # DMA issue order matches need order: first wq chunk + xt0 halves,
        # so the first qkv matmuls can start as early as possible.
        xt0 = p1x.tile([128, DCH, SPAN1], BF16, tag="xt")
        nc.sync.dma_start(
            wq_sb[:, :, 0:128],
            wq_d[:].rearrange("(c p) f -> p c f", p=128)[:, :, 0:128],
        )
        nc.sync.dma_start(xt0[:, 0:4, :], xt_view[:, 0:4, 0:SPAN1])
        nc.sync.dma_start(xt0[:, 4:8, :], xt_view[:, 4:8, 0:SPAN1])
        for fb in range(1, NFB):
            nc.sync.dma_start(
                wq_sb[:, :, fb * 128:(fb + 1) * 128],
                wq_d[:].rearrange("(c p) f -> p c f", p=128)[:, :, fb * 128:(fb + 1) * 128],
            )
        nc.sync.dma_start(cs_sb[:], cs_d[:])
        nc.sync.dma_start(sn_sb[:], sn_d[:])
        for fb in range(NFB):
            nc.sync.dma_start(
                wk_sb[:, :, fb * 128:(fb + 1) * 128],
                wk_d[:].rearrange("(c p) f -> p c f", p=128)[:, :, fb * 128:(fb + 1) * 128],
            )
        nc.sync.dma_start(wv_sb[:], wv_d[:].rearrange("(c p) f -> p c f", p=128))
        nc.sync.dma_start(tri_sb[:], tri_d[:])
        nc.sync.dma_start(trg_sb[:], trg_d[:])
        nc.sync.dma_start(id8_sb[:], id8_d[:])
        nc.sync.dma_start(idb_sb[:], idb_d[:])
        nc.sync.dma_start(wo_sb[:], wo_d[:].rearrange("(c p) d -> p c d", p=128))

        # ---- phase 1: qkv projection + RoPE ------------------------------
        def p1_gen(s1, xt=None):
            sl = slice(s1 * SPAN1, (s1 + 1) * SPAN1)
            if xt is None:
                if s1 == 0:
                    xt = xt0
                else:
                    xt = p1x.tile([128, DCH, SPAN1], BF16, tag="xt")
                    nc.sync.dma_start(xt[:], xt_view[:, :, sl])
            qr = prope.tile([128, 2, NFB, SPAN1], BF16, tag="qr")
            qs = prope.tile([128, 2, NFB, SPAN1], BF16, tag="qs")
            # q/k feature-block psums, two blocks per 1-bank tile
            for qk, wsb in ((0, wq_sb), (1, wk_sb)):
                for half in range(2):
                    # single start=True per psum tile (bank): the second
                    # feature-block's writes land on the armed pending-zero.
                    ps = pwork.tile([128, 2 * SPAN1], F32, tag="pw")
                    for fb2 in range(2):
                        fb = half * 2 + fb2
                        for c in range(DCH):
                            nc.tensor.matmul(
                                ps[:, fb2 * SPAN1:(fb2 + 1) * SPAN1],
                                wsb[:, c, fb * 128:(fb + 1) * 128],
                                xt[:, c, :],
                                start=(c == 0 and fb2 == 0),
                                stop=(c == DCH - 1),
                                skip_group_check=True,
                            )
                    with nc.allow_low_precision(reason="bf16 rope"):
                        nc.vector.tensor_copy(
                            qr[:, qk, half * 2:half * 2 + 2, :],
                            ps[:].rearrange("p (a b) -> p a b", a=2),
                        )
                    if half == 1:
                        for r0, sr in ((0, 32), (32, 0), (64, 96), (96, 64)):
                            nc.sync.dma_start(qs[r0:r0 + 32, qk, :, :],
                                              qr[sr:sr + 32, qk, :, :])
                    yield

            csl = cs_sb[:, sl]
            snl = sn_sb[:, sl]
            csb = bass.AP(csl.tensor, csl.offset,
                          [csl.ap[0], [0, NFB], csl.ap[-1]])
            snb = bass.AP(snl.tensor, snl.offset,
                          [snl.ap[0], [0, NFB], snl.ap[-1]])
            with nc.allow_low_precision(reason="bf16 rope"):
                # per-q/k ops: shorter chain to the consumers of qT/kTb
                nc.vector.tensor_mul(qs[:, 0], qs[:, 0], snb)
                yield
                nc.vector.tensor_mul(qr[:, 0], qr[:, 0], csb)
                yield
                nc.vector.tensor_add(qT[:, :, sl], qr[:, 0], qs[:, 0])
                yield
                nc.vector.tensor_mul(qs[:, 1], qs[:, 1], snb)
                yield
                nc.vector.tensor_mul(qr[:, 1], qr[:, 1], csb)
                yield
                nc.vector.tensor_add(kTb[:, :, sl], qr[:, 1], qs[:, 1])
                yield
            # V in [tok, feat] layout, evicted straight to fp8
            for tt in range(SPAN1 // 128):
                ktile = s1 * (SPAN1 // 128) + tt
                pv = pwork.tile([128, FEAT], F32, tag="pw")
                for c in range(DCH):
                    nc.tensor.matmul(
                        pv[:],
                        xt[:, c, tt * 128:(tt + 1) * 128],
                        wv_sb[:, c, :],
                        start=(c == 0),
                        stop=(c == DCH - 1),
                    )
                with nc.allow_low_precision(reason="fp8 V"):
                    nc.vector.tensor_copy(
                        v_sb[:, ktile, :, 0:HEAD_DIM],
                        pv[:].rearrange("p (h d) -> p h d", d=HEAD_DIM),
                    )
                    nc.vector.scalar_tensor_tensor(
                        v_lo[:, ktile, :, 0:HEAD_DIM],
                        pv[:].rearrange("p (h d) -> p h d", d=HEAD_DIM),
                        1.0, v_sb[:, ktile, :, 0:HEAD_DIM],
                        mybir.AluOpType.mult, mybir.AluOpType.subtract)
                yield

        # ---- attention -----------------------------------------------
        def lo_of(s, j):
            return max(0, (j - s * 4) * 128)

        def produce(pairs, at_buf, idx):
            h, s, ja = pairs[idx]
            hrow = 64 * (h % 2)
            hc = h // 2
            ps = pscore.tile([128, 2, SPAN2], F32, tag="ps_s")
            at = pat.tile([128, 2, SPAN2], FP8, tag="at")
            lo_a = lo_of(s, ja)
            for i, j in enumerate((ja, ja + 1)):
                lo = lo_of(s, j)
                nc.tensor.matmul(
                    ps[:, i, lo:],
                    kTb[hrow:hrow + 64, hc, j * 128:(j + 1) * 128],
                    qT[hrow:hrow + 64, hc, s * SPAN2 + lo:(s + 1) * SPAN2],
                    start=True, stop=True,
                )
            if ja >= 4 * s:  # diagonal pair: mask on the PE (fp8 DR: id8z
                # is [I | 0] so the dummy stride-0 rhs pair dim contributes 0)
                m = (ja - 4 * s) // 2
                c0 = 256 * m

                def mask_mm(out_ap, mask):
                    rhs = bass.AP(mask.tensor, mask.offset,
                                  [mask.ap[0], [0, 2], mask.ap[-1]])
                    nc.tensor.matmul(out_ap, id8_sb[:], rhs, start=False,
                                     stop=True, perf_mode=DR,
                                     skip_group_check=True)

                # half-0 triangle (accumulate -240 above-diagonal)
                mask_mm(ps[:, 0, c0:c0 + 128], tri_sb)
                # half-1 gap: full -240 lands on the bank's pending-zero
                mask_mm(ps[:, 1, c0:c0 + 128], trg_sb)
                # half-1 triangle
                if c0 + 256 <= SPAN2:
                    mask_mm(ps[:, 1, c0 + 128:c0 + 256], tri_sb)
            with nc.allow_low_precision(reason="fp8 softmax weights"):
                nc.scalar.activation(
                    at[:, :, lo_a:], ps[:, :, lo_a:],
                    mybir.ActivationFunctionType.Exp,
                    scale=float(1.0 / np.sqrt(HEAD_DIM)),
                    bias=bias_sb[:],
                )
            if debug and (h, s, ja) == (1, 0, 0):
                nc.sync.dma_start(dbg["at"][:], at[:])
                pscp = psc.tile([128, 2, SPAN2], F32, tag="pscp")
                nc.vector.tensor_copy(pscp[:], ps[:])
                nc.sync.dma_start(dbg["ps"][:], pscp[:])
            if debug and (h, s, ja) == (1, 0, 2):
                nc.sync.dma_start(dbg["at2"][:], at[:])
            at_buf[idx] = at

        def attn_span(s, avn, weave=None, k=1):
            pairs = []
            for h in range(HPC):
                for ja in range(0, 4 * s + 4, 2):
                    pairs.append((h, s, ja))
            at_buf = {}
            LOOKAHEAD = 2
            for i in range(min(LOOKAHEAD, len(pairs))):
                produce(pairs, at_buf, i)
            av = None
            for idx, (h, s_, ja) in enumerate(pairs):
                if idx + LOOKAHEAD < len(pairs):
                    produce(pairs, at_buf, idx + LOOKAHEAD)
                if weave is not None:
                    for _ in range(k):
                        next(weave, None)
                if ja == 0:
                    av = pav.tile([128, 4, HEAD_DIM + 1], F32, tag="ps_a")
                at = at_buf.pop(idx)
                m = (ja - 4 * s) // 2 if ja >= 4 * s else -1
                qc_lo = max(0, 2 * m)
                for qc in range(qc_lo, 4):
                    # one start=True per av tile: it arms the whole bank's
                    # pending-zero, so the other qc slots' first start=False
                    # writes accumulate from zero. Second matmul adds the
                    # fp8-residual of V.
                    last_ja = (4 * s + 2) if qc >= 2 else (4 * s)
                    nc.tensor.matmul(
                        av[:, qc, :],
                        at[:, :, qc * 128:(qc + 1) * 128],
                        v_sb[:, ja:ja + 2, h, :],
                        start=(ja == 0 and qc == 0), stop=False,
                        perf_mode=DR, skip_group_check=True,
                    )
                    nc.tensor.matmul(
                        av[:, qc, :],
                        at[:, :, qc * 128:(qc + 1) * 128],
                        v_lo[:, ja:ja + 2, h, :],
                        start=False, stop=(ja == last_ja),
                        perf_mode=DR, skip_group_check=True,
                    )
                if ja == 4 * s + 2:
                    # head eviction: normalize by softmax denominator
                    rc = psc.tile([128, 4], F32, tag="rc")
                    nc.vector.reciprocal(rc[:], av[:, :, HEAD_DIM])
                    with nc.allow_low_precision(reason="bf16 attn out"):
                        for qc in range(4):
                            nc.vector.tensor_scalar_mul(
                                avn[:, qc, h, :], av[:, qc, 0:HEAD_DIM],
                                rc[:, qc:qc + 1],
                            )

        def transpose_gen(s, avn):
            # attn^T tiles for span s: [128 (2 heads), 128 q] per (c, qc)
            for c in range(NFB):
                for qc in range(4):
                    pt = pwork.tile([128, 128], BF16, tag="pw")
                    nc.tensor.transpose(pt[:], avn[:, qc, 2 * c:2 * c + 2, :],
                                        idb_sb[:])
                    with nc.allow_low_precision(reason="bf16 attnT"):
                        nc.vector.tensor_copy(
                            attnT[:, c, s * SPAN2 + qc * 128:s * SPAN2 + (qc + 1) * 128],
                            pt[:],
                        )
                    yield

        # ---- output projection ------------------------------------------
        def proj_gen(tts):
            for tt in tts:
                for half in range(2):
                    po = pwork.tile([128, 512], F32, tag="pw")
                    for c in range(NFB):
                        nc.tensor.matmul(
                            po[:],
                            attnT[:, c, tt * 128:(tt + 1) * 128],
                            wo_sb[:, c, half * 512:(half + 1) * 512],
                            start=(c == 0),
                            stop=(c == NFB - 1),
                        )
                    ot = pot.tile([128, 512], F32, tag="ot")
                    nc.vector.tensor_copy(ot[:], po[:])
                    nc.sync.dma_start(
                        out_d[tt * 128:(tt + 1) * 128, half * 512:(half + 1) * 512],
                        ot[:],
                    )
                yield

        # ---- schedule ---------------------------------------------------
        from itertools import chain as _chain

        def run_gen(g):
            for _ in g:
                pass

        run_gen(p1_gen(0))
        run_gen(p1_gen(1))
        avns = {}
        for s in range(NSPAN2):
            avn_t = pavn.tile([128, 4, HPC, HEAD_DIM], BF16, tag="avn")
            avns[s] = avn_t
            gens = []
            if s > 0:
                gens.append(transpose_gen(s - 1, avns[s - 1]))
            if s < NSPAN2 - 1:
                for sp in (2 * s + 2, 2 * s + 3):
                    if sp < NSPAN1:
                        xt_pf = p1x.tile([128, DCH, SPAN1], BF16, tag="xt")
                        nc.sync.dma_start(
                            xt_pf[:], xt_view[:, :, sp * SPAN1:(sp + 1) * SPAN1]
                        )
                        gens.append(p1_gen(sp, xt=xt_pf))
            else:
                gens.append(proj_gen(range(4 * (NSPAN2 - 1))))
            w = _chain(*gens)
            attn_span(s, avns[s], weave=w, k=2 if s == 0 else 1)
            for _ in w:
                pass
            if debug and s == 0:
                nc.sync.dma_start(dbg["avn"][:], avns[0][:])
        # tail: last span's transposes + its projection
        run_gen(transpose_gen(NSPAN2 - 1, avns[NSPAN2 - 1]))
        run_gen(proj_gen(range(4 * (NSPAN2 - 1), NTOK)))

        if debug:
            nc.sync.dma_start(dbg["qT"][:], qT[:])
            nc.sync.dma_start(dbg["kT"][:], kTb[:])
            nc.sync.dma_start(dbg["v"][:], v_sb[:])
            nc.sync.dma_start(dbg["attnT"][:], attnT[:])

        for p in reversed(pools):
            p.release()
        pools.clear()

    nc.finalize()
    return nc


# ---------------------------------------------------------------------------
# Host-side input prep

def _np(dt):
    return mybir.dt.np(dt)


def rope_tables(T, dtype):
    inv_freq = 1.0 / (
        ROPE_BASE ** (np.arange(0, HEAD_DIM, 2, dtype=np.float64) / HEAD_DIM)
    )
    freqs = np.arange(T, dtype=np.float64)[:, None] * inv_freq[None, :]  # [T, 32]
    emb = np.concatenate([freqs, freqs], axis=-1)  # [T, 64]
    cos = np.cos(emb).T  # [64, T]
    sin = np.sin(emb).T
    cs = np.tile(cos, (2, 1)).astype(dtype)  # [128, T]
    sn_half = np.concatenate([-sin[:32], sin[32:]], axis=0)  # sign folded in
    sn = np.tile(sn_half, (2, 1)).astype(dtype)
    return np.ascontiguousarray(cs), np.ascontiguousarray(sn)


def _wo_hilo(wo, f8):
    hi = wo.astype(f8)
    lo = (wo - hi.astype(np.float64)).astype(f8)
    return np.ascontiguousarray(np.stack([hi, lo], axis=1))


def make_core_inputs(x, Wqkv, Wout, T=T_FULL):
    bf = _np(BF16)
    f8 = _np(FP8)
    cs, sn = rope_tables(T, bf)
    r = np.arange(128)
    tri = (MASKVAL * (r[None, :] < r[:, None])).astype(f8)     # -240 below col<row
    trg = np.full((128, 128), MASKVAL, dtype=np.float64).astype(f8)
    id8 = np.zeros((128, 2, 128), dtype=f8)
    id8[:, 0, :] = np.eye(128).astype(f8)
    idb = np.eye(128).astype(bf)

    in_maps = []
    for core in range(N_CORES):
        b, g = divmod(core, 2)
        in_maps.append(
            {
                "xt": np.ascontiguousarray(x[b].T).astype(bf),
                "wq": np.ascontiguousarray(Wqkv[:, g * FEAT:(g + 1) * FEAT]).astype(bf),
                "wk": np.ascontiguousarray(
                    Wqkv[:, D_MODEL + g * FEAT:D_MODEL + (g + 1) * FEAT]
                ).astype(bf),
                "wv": np.ascontiguousarray(
                    Wqkv[:, 2 * D_MODEL + g * FEAT:2 * D_MODEL + (g + 1) * FEAT]
                ).astype(bf),
                "wo": np.ascontiguousarray(Wout[g * FEAT:(g + 1) * FEAT, :]).astype(bf),
                "cs": cs,
                "sn": sn,
                "tri": tri,
                "trg": trg,
                "id8": id8,
                "idb": idb,
            }
        )
    return in_maps


_NC_CACHE = {}


def get_nc(T=T_FULL):
    if T not in _NC_CACHE:
        _NC_CACHE[T] = build_nc(T)
    return _NC_CACHE[T]


def kernel(x, Wqkv, Wout):
    x = np.asarray(x, dtype=np.float32)
    Wqkv = np.asarray(Wqkv, dtype=np.float32)
    Wout = np.asarray(Wout, dtype=np.float32)
    b, t, _ = x.shape
    assert (b, t) == (BATCH, T_FULL)

    nc = get_nc(T_FULL)
    in_maps = make_core_inputs(x, Wqkv, Wout, T_FULL)
    res = None
    for attempt in range(3):
        try:
            res = run_bass_kernel_spmd(nc, in_maps, core_ids=list(range(N_CORES)))
            break
        except Exception:
            if attempt == 2:
                raise
            import time

            time.sleep(5.0)
    out = np.empty((BATCH, T_FULL, D_MODEL), dtype=np.float32)
    for bb in range(BATCH):
        out[bb] = res.results[2 * bb]["out"] + res.results[2 * bb + 1]["out"]
    return out
